# revision 51
# baseline (speedup 1.0000x reference)
"""DenseGAT layer kernel for 8 Trainium2 NeuronCores (Bass/Tile), v16.

Math: 3-term separable approximation of the kinked exponential,
    exp(leaky_relu(e,0.2)) ~= e^e + e^{0.2e} - 0.75*e^{0.52e},
exact in both tails; softmax cancels per-query constants. Each term is
rank-1 over (query, key), so masked-softmax attention becomes three PE
matmuls with the fp8 adjacency as the stationary operand - no N^2
elementwise work:
    A_m[q,(d,h)|den] = sum_j adj[j,q] * (Bm_j*h_j | Bm_j)
    o[q] = (r1*A1 + A2 - r3*A3) / (r1*D1 + D2 - r3*D3)
with Bm = e^{gm*d + bB_m}, r_m = e^{(gm-0.2)s + br_m}.

v16 highlights:
 - m=3 correction channel (values+den) in fp8e4 with DoubleRow matmuls
   (two 128-key planes per instruction at 0.5 cyc/row). B3's bias is
   anchored per (core,head) on the host (biases ride wvsd row 0).
 - hidden dim is (d,h)-ordered so every per-head broadcast is
   contiguous-last: V12-mult and the combine run at DVE 2x_1p.
 - PSUM packing: per query-tile only TWO banks (A1|A2 nums packed in one,
   A3+dens in the other; one start per bank, per-element has_written) so
   THREE query tiles are in flight and the AV overlaps the DMA-paced
   production phase.
 - po tiles copied to bf16 SBUF by Act, combine on DVE at 2x; residual x
   injected into the Wo PSUM group via identity matmul; bf16 identity
   transposes; ph copied to bf16 SBUF by Act; V3-mult on Pool.
"""

import sys

sys.path.insert(0, "/opt/trn_rl_repo")

from contextlib import ExitStack

import ml_dtypes
import numpy as np

B, N, D, H = 4, 2048, 256, 4
DH = D // H
NQ = N // 2
NCORES = 8
LN_EPS = 1e-5
KT = D // 128
NT = 16
QT = 8
C12 = 25.0
C3COEF = 0.75
G3 = 0.52
F8_CAP_LN = float(np.log(240.0 / 2.0))
F8_DEN_CAP_LN = float(np.log(240.0 / 1.3))
F32 = np.float32
F16 = np.float16

LOOKAHEAD = 4

_BUILT = {}


def _build(skip_bo=False, skip_gamma=False, skip_beta=False):
    import concourse.bass as bass
    import concourse.mybir as mybir
    import concourse.tile as tile
    from concourse import bacc
    from concourse.masks import make_identity

    fp32 = mybir.dt.float32
    bf16 = mybir.dt.bfloat16
    fp16 = mybir.dt.float16
    f8e4 = mybir.dt.float8e4
    Alu = mybir.AluOpType
    Act = mybir.ActivationFunctionType
    DR = mybir.MatmulPerfMode.DoubleRow

    nc = bacc.Bacc(None, target_bir_lowering=False, debug=False)

    xT = nc.dram_tensor("xT", [D, N], fp16, kind="ExternalInput")
    xs = nc.dram_tensor("xs", [NQ, D], fp16, kind="ExternalInput")
    adjT = nc.dram_tensor("adjT", [N, NQ], f8e4, kind="ExternalInput")
    # cols [0:D]=W^T (d,h)-ordered, [D:D+20]=vsd, row 0 [D+20:D+40]=biases
    wvsd = nc.dram_tensor("wvsd", [D, D + 40], fp16, kind="ExternalInput")
    woT = nc.dram_tensor("woT", [D, D], fp16, kind="ExternalInput")
    bo = nc.dram_tensor("bo", [1, D], fp32, kind="ExternalInput")
    gamma = nc.dram_tensor("gamma", [1, D], fp32, kind="ExternalInput")
    beta = nc.dram_tensor("beta", [1, D], fp32, kind="ExternalInput")
    out = nc.dram_tensor("out", [NQ, D], fp16, kind="ExternalOutput")

    with tile.TileContext(nc) as tc, ExitStack() as ctx:
        singles = ctx.enter_context(tc.tile_pool(name="singles", bufs=1))
        work = ctx.enter_context(tc.tile_pool(name="work", bufs=8))
        phwork = ctx.enter_context(tc.tile_pool(name="phwork", bufs=6))
        small = ctx.enter_context(tc.tile_pool(name="small", bufs=10))
        p_acc = ctx.enter_context(tc.tile_pool(name="p_acc", bufs=3, space="PSUM"))
        p_ph = ctx.enter_context(tc.tile_pool(name="p_ph", bufs=2, space="PSUM"))

        def bcast_row(row_ap, parts=128):
            return bass.AP(
                tensor=row_ap.tensor,
                offset=row_ap.offset,
                ap=[[0, parts]] + [list(d) for d in row_ap.ap[1:]],
            )

        def ap_with(src_ap, pattern):
            return bass.AP(tensor=src_ap.tensor, offset=src_ap.offset, ap=pattern)

        # ---- consts ----
        eps_sb = singles.tile([128, 1], fp32, tag="eps")
        nc.gpsimd.memset(eps_sb, LN_EPS)
        ones1 = singles.tile([1, 128], fp16, tag="ones1")
        nc.gpsimd.memset(ones1, 1.0)
        identb = singles.tile([128, 128], bf16, tag="identb")
        make_identity(nc, identb)

        # ---- DMAs ordered by first need ----
        wvsd_sb = singles.tile([128, KT, D + 40], fp16, tag="wvsd")
        wT_sb = wvsd_sb[:, :, 0:D]
        vsd_sb = wvsd_sb[:, :, D : D + 20]
        brow_sb = wvsd_sb[0:1, 0, D + 20 : D + 40]
        xT_sb = singles.tile([128, KT, N], fp16, tag="xT")
        adj_sb = singles.tile([128, NT, NQ], f8e4, tag="adj")
        xT_r = xT.rearrange("(k p) n -> p k n", p=128)
        adj_r = adjT.rearrange("(t p) q -> p t q", p=128)
        nc.sync.dma_start(out=xT_sb[:, :, 0:256], in_=xT_r[:, :, 0:256])
        nc.sync.dma_start(
            out=wvsd_sb, in_=wvsd.rearrange("(k p) d -> p k d", p=128)
        )
        warm = small.tile([1, 1], fp32, tag="warm")
        nc.gpsimd.memset(warm, 0.0)
        nc.scalar.activation(out=warm, in_=warm, func=Act.Exp)
        nc.sync.dma_start(out=xT_sb[:, :, 256:1024], in_=xT_r[:, :, 256:1024])
        nc.sync.dma_start(out=adj_sb[:, 0:2, :], in_=adj_r[:, 0:2, :])
        nc.sync.dma_start(out=adj_sb[:, 2:8, :], in_=adj_r[:, 2:8, :])
        nc.sync.dma_start(out=xT_sb[:, :, 1024:2048], in_=xT_r[:, :, 1024:2048])
        nc.sync.dma_start(out=adj_sb[:, 8:16, :], in_=adj_r[:, 8:16, :])
        woT_sb = singles.tile([128, KT, D], fp16, tag="woT")
        nc.sync.dma_start(out=woT_sb, in_=woT.rearrange("(k p) d -> p k d", p=128))
        xs_sb = singles.tile([128, QT, D], fp16, tag="xs")
        nc.sync.dma_start(out=xs_sb, in_=xs.rearrange("(t p) d -> p t d", p=128))
        if not skip_bo:
            bo_bc = singles.tile([128, D], fp32, tag="bo")
            nc.sync.dma_start(out=bo_bc, in_=bcast_row(bo[:, :]))
        if not skip_gamma:
            gamma_bc = singles.tile([128, D], fp32, tag="gamma")
            nc.sync.dma_start(out=gamma_bc, in_=bcast_row(gamma[:, :]))
        if not skip_beta:
            beta_bc = singles.tile([128, D], fp32, tag="beta")
            nc.sync.dma_start(out=beta_bc, in_=bcast_row(beta[:, :]))

        # ---- per-tile production ----
        FD = D + 4  # A-tile width: 256 nums + 4 dens
        V12 = singles.tile([128, NT, 2, FD], bf16, tag="V12")
        V2f = singles.tile([128, NT, D], f8e4, tag="V2f")
        V3 = singles.tile([128, NT, FD], f8e4, tag="V3")
        # RB[t, 0:8] = r-cols (only valid for t<QT), RB[t, 8:20] = B_m
        RB = singles.tile([128, NT, 20], bf16, tag="RB")

        def xsl(k, t):
            return xT_sb[:, k, t * 128 : (t + 1) * 128]

        def emit_prod(t):
            phsd = p_ph.tile([128, D + 20], fp32, tag="ph", name=f"ph{t}")
            ph, psd = phsd[:, 0:D], phsd[:, D : D + 20]
            for k in range(KT):
                nc.tensor.matmul(
                    psd, lhsT=xsl(k, t), rhs=vsd_sb[:, k, :],
                    start=(k == 0), stop=False,
                )
            nc.tensor.matmul(psd, lhsT=ones1, rhs=brow_sb, start=False, stop=False)
            for k in range(KT):
                nc.tensor.matmul(
                    ph, lhsT=xsl(k, t), rhs=wT_sb[:, k, :],
                    start=False, stop=(k == KT - 1),
                )
            if t < QT:
                nc.scalar.activation(
                    out=RB[:, t, 0:20], in_=psd[:, 0:20], func=Act.Exp
                )
            else:
                nc.scalar.activation(
                    out=RB[:, t, 8:20], in_=psd[:, 8:20], func=Act.Exp
                )
            ph_sb = phwork.tile([128, D], bf16, tag="phsb", name=f"phsb{t}")
            nc.scalar.copy(out=ph_sb, in_=ph)
            b = RB[:, t, 8:20]
            # denominator columns: copies of B_m into V-tile cols [256:260]
            nc.vector.tensor_copy(
                out=V12[:, t, :, D:FD],
                in_=ap_with(b, [list(b.ap[0]), [4, 2], [1, 4]]),
            )
            dv3in = ap_with(b, [list(b.ap[0]), [1, 4]])
            dv3in.offset += 8
            nc.vector.tensor_copy(out=V3[:, t, D:FD], in_=dv3in)
            # m=1 numerator (bf16, 2x_1p: all operands 2-byte, contig-last)
            in1 = ap_with(b, [list(b.ap[0]), [0, DH], [1, 4]])
            nc.vector.tensor_tensor(
                out=V12[:, t, 0, 0:D], in0=ph_sb[:, :], in1=in1, op=Alu.mult
            )
            # m=2 numerator (f8e4) on DVE
            in1a = ap_with(b, [list(b.ap[0]), [0, DH], [1, 4]])
            in1a.offset += 4
            nc.vector.tensor_tensor(
                out=V2f[:, t, :], in0=ph_sb[:, :], in1=in1a, op=Alu.mult
            )
            # m=3 numerator (f8e4) on Pool
            in1b = ap_with(b, [list(b.ap[0]), [0, DH], [1, 4]])
            in1b.offset += 8
            nc.gpsimd.tensor_tensor(
                out=V3[:, t, 0:D], in0=ph_sb[:, :], in1=in1b, op=Alu.mult
            )

        # ---- attention ----
        o_sb = singles.tile([128, QT, D], bf16, tag="o_sb")
        outT = singles.tile([128, KT, NQ], fp16, tag="outT")
        po = {}

        def v12den(t):
            v = V12[:, t, 0]
            a = ap_with(v, [list(v.ap[0]), [FD, 2], [1, 4]])
            a.offset += D
            return a

        def emit_av_t(qts, t):
            for qt in qts:
                qs = slice(qt * 128, (qt + 1) * 128)
                st, sp = (t == 0), (t == NT - 1)
                pa, pb = po[qt]
                # Two packed streams per bank. HW: start=True clears
                # has_written for the WHOLE bank; start=False writes
                # overwrite-and-set where clear, accumulate where set. So
                # exactly one start per bank: m1 for bank a, den for bank b.
                nc.tensor.matmul(
                    pa[:, 0:D], lhsT=adj_sb[:, t, qs], rhs=V12[:, t, 0, 0:D],
                    start=st, stop=sp, skip_group_check=True,
                )
                if t % 2 == 1:
                    nc.tensor.matmul(
                        pa[:, D : 2 * D],
                        lhsT=adj_sb[:, t - 1 : t + 1, qs],
                        rhs=V2f[:, t - 1 : t + 1],
                        start=False, stop=sp, perf_mode=DR,
                        skip_group_check=True,
                    )
                nc.tensor.matmul(
                    pb[:, FD : FD + 8], lhsT=adj_sb[:, t, qs], rhs=v12den(t),
                    start=st, stop=sp, skip_group_check=True,
                )
                if t % 2 == 1:
                    nc.tensor.matmul(
                        pb[:, 0:FD],
                        lhsT=adj_sb[:, t - 1 : t + 1, qs],
                        rhs=V3[:, t - 1 : t + 1],
                        start=False,
                        stop=sp,
                        perf_mode=DR,
                        skip_group_check=True,
                    )

        def alloc_po(qt):
            po[qt] = tuple(
                p_acc.tile([128, 512], fp32, tag=f"po{m}", name=f"po{qt}_{m}")
                for m in range(2)
            )

        def emit_av(qts):
            for qt in qts:
                alloc_po(qt)
            for t in range(NT):
                emit_av_t(qts, t)

        def emit_combine(qt, direct=False):
            pa, pb = po[qt]
            if direct:
                # low-latency variant for the last tile: DVE reads PSUM
                # fp32 directly (no Act copy hop, no 2x)
                pa_sb, pb_sb = pa, pb
            else:
                # Act copies PSUM -> bf16 SBUF so DVE runs 2x_1p
                pa_sb = work.tile(
                    [128, 2 * D], bf16, tag="pasb", name=f"pasb{qt}"
                )
                nc.scalar.copy(out=pa_sb, in_=pa[:, 0 : 2 * D])
                pb_sb = work.tile(
                    [128, FD + 8], bf16, tag="pbsb", name=f"pbsb{qt}"
                )
                nc.scalar.copy(out=pb_sb, in_=pb[:, 0 : FD + 8])

            def rbc(lo):
                r = RB[:, qt, lo : lo + 4]
                return ap_with(r, [list(r.ap[0]), [0, DH], [1, 4]])

            t1f = work.tile([128, D], bf16, tag="t1f", name=f"t1f{qt}")
            nc.vector.tensor_tensor(
                out=t1f, in0=pa_sb[:, 0:D], in1=rbc(0), op=Alu.mult
            )
            t2f = work.tile([128, D], bf16, tag="t2f", name=f"t2f{qt}")
            nc.vector.tensor_tensor(
                out=t2f, in0=pa_sb[:, D : 2 * D], in1=t1f, op=Alu.add
            )
            t3f = work.tile([128, D], bf16, tag="t3f", name=f"t3f{qt}")
            nc.vector.tensor_tensor(
                out=t3f, in0=pb_sb[:, 0:D], in1=rbc(4), op=Alu.mult
            )
            nnegf = work.tile([128, D], bf16, tag="nnegf", name=f"nnegf{qt}")
            nc.vector.tensor_tensor(out=nnegf, in0=t3f, in1=t2f, op=Alu.subtract)
            # dens: [D3 | D1 | D2] at pb_sb[256:268]
            t1d = small.tile([128, 4], bf16, tag="t1d", name=f"t1d{qt}")
            nc.vector.tensor_tensor(
                out=t1d, in0=pb_sb[:, FD : FD + 4], in1=RB[:, qt, 0:4],
                op=Alu.mult,
            )
            t2d = small.tile([128, 4], bf16, tag="t2d", name=f"t2d{qt}")
            nc.vector.tensor_tensor(
                out=t2d, in0=pb_sb[:, FD + 4 : FD + 8], in1=t1d, op=Alu.add
            )
            t3d = small.tile([128, 4], bf16, tag="t3d", name=f"t3d{qt}")
            nc.vector.tensor_tensor(
                out=t3d, in0=pb_sb[:, D:FD], in1=RB[:, qt, 4:8], op=Alu.mult
            )
            dneg = small.tile([128, 4], fp32, tag="dneg", name=f"dneg{qt}")
            nc.vector.tensor_tensor(out=dneg, in0=t3d, in1=t2d, op=Alu.subtract)
            rdn = small.tile([128, 4], fp32, tag="rdn", name=f"rdn{qt}")
            nc.vector.reciprocal(out=rdn, in_=dneg)
            rdnb = small.tile([128, 4], bf16, tag="rdnb", name=f"rdnb{qt}")
            nc.vector.tensor_copy(out=rdnb, in_=rdn)
            rdnv = ap_with(rdnb[:, :], [list(rdnb.ap[0]), [0, DH], [1, 4]])
            nc.vector.tensor_tensor(
                out=o_sb[:, qt, :], in0=nnegf, in1=rdnv, op=Alu.mult
            )

        def emit_tail(qt, last=False):
            ptr = p_ph.tile([128, KT, 128], bf16, tag="ph", name=f"tr{qt}")
            for k in range(KT):
                nc.tensor.transpose(
                    out=ptr[:, k, :], in_=o_sb[:, qt, k * 128 : (k + 1) * 128],
                    identity=identb,
                )
            oT = ap_with(
                outT[:, 0, qt * 128 : (qt + 1) * 128],
                [list(outT.ap[0]), [NQ, KT], [1, 128]],
            )
            if last:
                nc.vector.tensor_copy(out=oT, in_=ptr)
            else:
                nc.scalar.copy(out=oT, in_=ptr)
            pp = p_ph.tile([128, D], fp32, tag="ph", name=f"pp{qt}")
            for k in range(KT):
                nc.tensor.matmul(
                    pp, lhsT=outT[:, k, qt * 128 : (qt + 1) * 128],
                    rhs=woT_sb[:, k, :], start=(k == 0), stop=False,
                )
            nc.tensor.matmul(
                pp, lhsT=identb, rhs=xs_sb[:, qt, :], start=False, stop=True
            )
            y = pp
            if not skip_bo:
                yb = work.tile([128, D], fp32, tag="yb", name=f"yb{qt}")
                nc.vector.tensor_tensor(out=yb, in0=pp, in1=bo_bc, op=Alu.add)
                y = yb
            stats = small.tile([128, 6], fp32, tag="stats")
            nc.vector.bn_stats(out=stats, in_=y)
            mv = small.tile([128, 2], fp32, tag="mv")
            nc.vector.bn_aggr(out=mv, in_=stats)
            sq = small.tile([128, 1], fp32, tag="sq")
            nc.scalar.activation(
                out=sq, in_=mv[:, 1:2], func=Act.Sqrt, bias=eps_sb, scale=1.0
            )
            rstd = small.tile([128, 1], fp32, tag="rstd")
            nc.vector.reciprocal(out=rstd, in_=sq)
            xh = work.tile(
                [128, D], fp16 if (skip_gamma and skip_beta) else fp32,
                tag="xh", name=f"xh{qt}",
            )
            nc.vector.tensor_scalar(
                out=xh, in0=y, scalar1=mv[:, 0:1], scalar2=rstd,
                op0=Alu.subtract, op1=Alu.mult,
            )
            if not (skip_gamma and skip_beta):
                xh2 = work.tile([128, D], fp16, tag="xh2", name=f"xh2{qt}")
                if not skip_gamma:
                    nc.vector.tensor_tensor(out=xh, in0=xh, in1=gamma_bc, op=Alu.mult)
                if not skip_beta:
                    nc.vector.tensor_tensor(out=xh, in0=xh, in1=beta_bc, op=Alu.add)
                nc.vector.tensor_copy(out=xh2, in_=xh)
                xh = xh2
            nc.sync.dma_start(out=out[qt * 128 : (qt + 1) * 128, :], in_=xh)

        # ---- schedule ----
        alloc_po(0)
        alloc_po(1)
        alloc_po(2)
        for t in range(NT):
            emit_prod(t)
            if t >= LOOKAHEAD:
                emit_av_t((0, 1, 2), t - LOOKAHEAD)
        for t in range(NT - LOOKAHEAD, NT):
            emit_av_t((0, 1, 2), t)
        emit_combine(0)
        emit_combine(1)
        emit_tail(0)
        emit_combine(2)
        emit_tail(1)
        for qt in range(3, QT - 1):
            emit_av((qt,))
            emit_tail(qt - 1)
            emit_combine(qt)
        emit_av((QT - 1,))
        emit_combine(QT - 1, direct=True)
        emit_tail(QT - 2)
        emit_tail(QT - 1, last=True)

    nc.finalize()
    return nc


def _host_prep(inputs):
    x = np.asarray(inputs["x"], F32)
    adj = np.asarray(inputs["adj"])
    W = np.asarray(inputs["W"], F32)
    a_src = np.asarray(inputs["a_src"], F32)
    a_dst = np.asarray(inputs["a_dst"], F32)
    Wo = np.asarray(inputs["Wo"], F32)
    bo = np.asarray(inputs["bo"], F32).reshape(1, D)
    gamma = np.asarray(inputs["gamma"], F32).reshape(1, D)
    beta = np.asarray(inputs["beta"], F32).reshape(1, D)
    f8 = ml_dtypes.float8_e4m3

    V_dst = np.stack([a_dst[h] @ W[h * DH : (h + 1) * DH, :] for h in range(H)], 1)
    V_src = np.stack([a_src[h] @ W[h * DH : (h + 1) * DH, :] for h in range(H)], 1)
    vsd = np.concatenate(
        [0.8 * V_src, (G3 - 0.2) * V_src, V_dst, 0.2 * V_dst, G3 * V_dst], axis=1
    ).astype(F32)

    # (d,h) permutation: position j = d*H + h holds standard channel h*DH+d
    perm_dh = (np.arange(D) % H) * DH + (np.arange(D) // H)
    wT = np.ascontiguousarray(W.T[:, perm_dh])
    woT = np.ascontiguousarray(Wo.T[perm_dh, :]).astype(F16)

    # per-batch B3 bias anchor: cap |B3*h| products at 240/2 and B3 itself
    # (den col) at 240/1.3
    brows = []
    for b in range(B):
        hb = x[b] @ W.T  # [N, D]
        d_all = x[b] @ V_dst  # [N, H]
        lnh_max = np.log(np.abs(hb.reshape(N, H, DH)).max(-1) + 1e-30)  # [N,H]
        bB3 = np.minimum(
            F8_CAP_LN - (G3 * d_all + lnh_max).max(0),
            F8_DEN_CAP_LN - G3 * d_all.max(0),
        )  # [H]
        # quantize bB3 to fp16 first so br3 cancels the quantized value
        bB3 = np.asarray(bB3, F16).astype(F32)
        bB2 = np.asarray(
            F8_CAP_LN - (0.2 * d_all + lnh_max).max(0), F16
        ).astype(F32)
        br3 = np.log(C3COEF) + bB2 - bB3
        brow = np.empty(20, F32)
        brow[0:4] = bB2 + C12
        brow[4:8] = br3
        brow[8:12] = -C12
        brow[12:16] = bB2
        brow[16:20] = bB3
        wv = np.zeros((D, D + 40), F32)
        wv[:, 0:D] = wT
        wv[:, D : D + 20] = vsd
        wv[0, D + 20 : D + 40] = brow
        brows.append(np.ascontiguousarray(wv).astype(F16))

    in_maps = []
    for c in range(NCORES):
        b, half = divmod(c, 2)
        i0 = half * NQ
        perm = np.concatenate(
            [np.arange(i0, i0 + NQ), np.arange(0, i0), np.arange(i0 + NQ, N)]
        )
        xb = x[b]
        in_maps.append(
            {
                "xT": np.ascontiguousarray(xb[perm].T).astype(F16),
                "xs": np.ascontiguousarray(xb[i0 : i0 + NQ]).astype(F16),
                "adjT": np.ascontiguousarray(adj[i0 : i0 + NQ, perm].T).astype(f8),
                "wvsd": brows[b],
                "woT": woT,
                "bo": bo,
                "gamma": gamma,
                "beta": beta,
            }
        )
    return in_maps


def kernel(**inputs) -> np.ndarray:
    from concourse.bass_utils import run_bass_kernel_spmd

    flags = (
        bool(np.all(np.asarray(inputs["bo"]) == 0.0)),
        bool(np.all(np.asarray(inputs["gamma"]) == 1.0)),
        bool(np.all(np.asarray(inputs["beta"]) == 0.0)),
    )
    if flags not in _BUILT:
        _BUILT[flags] = _build(*flags)
    nc = _BUILT[flags]

    in_maps = _host_prep(inputs)
    res = run_bass_kernel_spmd(nc, in_maps, core_ids=list(range(NCORES)))
    full = np.empty((B, N, D), F32)
    for c in range(NCORES):
        b, half = divmod(c, 2)
        full[b, half * NQ : (half + 1) * NQ] = res.results[c]["out"].astype(F32)
    return full
